# revision 46
# baseline (speedup 1.0000x reference)
"""Trainium2 Bass kernel for nn_FDConv (per-sample frequency-domain-synthesized
3x3 grouped conv).

Strategy (data-parallel over batch, 1 sample per NeuronCore):
  - host: permute dft_weight into dense half-spectrum layout (pure gather),
    precompute phase-blocked DFT basis matrices as constants, replicate the
    16B logits to all partitions, stage x as bf16 in a padded parity-split
    layout so every load descriptor is one contiguous run.
  - device per core:
      warmup: 11 dummy matmuls so the PE HAM clock-gate releases (2.4 GHz)
      before the DFT chain instead of 20us into the conv; all later stage
      boundaries keep PE gaps well under the ~3us re-throttle window.
      att = sigmoid(logits)                  (per-partition, no broadcast;
                                              0.5 att scale folded in basis)
      attD = att_k * spectrum_k on DVE       (8 fast tensor_scalar ops,
                                              k-interleaved so stage-1
                                              matmuls pipeline behind them)
      GT  = sum_k attD_k^T . [C | S]         (stage-1 iFFT along axis 0,
            32 matmuls N=256; basis columns pre-arranged in phase blocks
            [ph1 ph0 ph2 ph1] so stage-2 rhs windows are contiguous)
      T[which,dx] = A_dx^T.GT_re - B_dx^T.GT_im  (stage-2 irfft: 2 matmul
            pairs per quadrant-merged target -> six 128x128 weight mats)
      conv: x bf16 with even rows on partitions 0-63 and odd rows on 64-127;
            two output row pairs per matmul group: six K=128/M=128/N=512
            matmuls accumulated in PSUM (75% PE utilization bound; the
            83.6us conv loop is at the N-column streaming floor).
  - one latency-critical load per DMA queue (pkA/sync, pkB/scalar,
    logits/gpsimd), all padded to 128 partitions (narrower transfers get
    straggler semaphore increments); x chunks gated behind them.
  - ONE unified 8-bank PSUM pool for warmup/DFT/conv so the conv's first
    accumulators reuse the earliest-freed banks (no WAR stall on T copies).
  - outputs copied PSUM->SBUF (DVE/ACT) as bf16 and stored with ONE
    128-partition DMA per slot-group (descgen on the engine queues is ~1us
    per dma_start, so few big DMAs beat many small ones); host reassembles
    the row interleave and upcasts to f32.
"""

import numpy as np
import ml_dtypes

import concourse.bass as bass
import concourse.bacc as bacc
import concourse.tile as tile
import concourse.mybir as mybir
from concourse.bass_utils import run_bass_kernel_spmd

F32 = mybir.dt.float32
BF16 = mybir.dt.bfloat16

B, CIN, COUT, KS = 8, 64, 64, 3
H, W = 256, 256
KNUM = 4
D1, D2 = COUT * KS, CIN * KS          # 192, 192
D2R = D2 // 2 + 1                     # 97
NF = D1 * D2R                         # 18624

NPAIR = 128          # output row pairs (2u+1, 2u+2), u = -1..127
SLOT = W + 2         # 258: [pad, 256 cols, pad] per row-slot
CHS = 16             # slots per x chunk
NCH = NPAIR // CHS   # 8 chunks

# constant packs (bf16, padded to 128 partitions):
#   pkA = [dre | dim]          width 1552   (spectrum, gates the att-scale)
#   pkB = [C | S | -S] blocks  width 1536   (basis, gates stage 1)
_PKAW = 2 * 8 * D2R           # 1552
_PKBW = 3 * 512               # 1536


def _phase_blocks(m):
    # [96, 192] -> [96, 256] blocks [ph1, ph0, ph2, ph1] so every stage-2
    # quadrant rhs is a contiguous window
    p = [m[:, c::3] for c in range(3)]
    return np.concatenate([p[1], p[0], p[2], p[1]], axis=1)


def _host_constants():
    fh = np.fft.fftfreq(D1)
    fw = np.fft.rfftfreq(D2)
    dist = np.sqrt(fh[:, None] ** 2 + fw[None, :] ** 2)
    idx = np.argsort(dist.ravel(), kind='stable')
    FH = (idx // D2R).astype(np.int64)
    FW = (idx % D2R).astype(np.int64)
    perm = FH * D2R + FW
    inv = np.empty(NF, dtype=np.int64)
    inv[perm] = np.arange(NF)

    hh = np.arange(D1)
    ang = 2.0 * np.pi * np.outer(hh, hh) / D1
    # att scale 2/KNUM = 0.5 folded into the stage-1 basis
    Cb = (np.cos(ang) * (0.5 / D1)).astype(np.float32)
    Sb = (np.sin(ang) * (0.5 / D1)).astype(np.float32)
    # per row-half, phase-blocked: [96, 2*256]
    C2 = np.concatenate([_phase_blocks(Cb[:96]), _phase_blocks(Cb[96:])], axis=1)
    S2 = np.concatenate([_phase_blocks(Sb[:96]), _phase_blocks(Sb[96:])], axis=1)

    w_ = np.arange(D2R)
    n_ = np.arange(D2)
    alpha = np.full(D2R, 2.0); alpha[0] = 1.0; alpha[-1] = 1.0
    beta = np.full(D2R, 2.0); beta[0] = 0.0; beta[-1] = 0.0
    ang2 = 2.0 * np.pi * np.outer(w_, n_) / D2
    A = (alpha[:, None] * np.cos(ang2) / D2).astype(np.float32)   # [97, 192]
    Bm = (beta[:, None] * np.sin(ang2) / D2).astype(np.float32)
    ab = np.concatenate(
        [A[:, dx::3] for dx in range(3)] + [-Bm[:, dx::3] for dx in range(3)],
        axis=1,
    ).astype(ml_dtypes.bfloat16)                               # [97, 384]
    return inv, C2, S2, ab


_INV, _C2, _S2, _AB = _host_constants()

# (which, dx) order used in the conv weight loop; t_sb index = 2*dx + which
_WSEQ = [(0, 0), (1, 0), (0, 1), (1, 1), (0, 2), (1, 2)]
_ZQUAD = {0: (0, 1), 1: (1, 0)}  # zero quadrant (J, r)
# stage-2 matmul plan per `which`: (J, out col range, rhs col range)
_S2PLAN = {
    0: [(1, 0, 128, 0, 128), (0, 0, 64, 64, 128)],
    1: [(0, 0, 128, 128, 256), (1, 64, 128, 128, 192)],
}


def _emit_kernel(tc):
    nc = tc.nc
    from contextlib import ExitStack

    # x: [(parity,cin)=128, slot, 258] bf16, host-padded (col 0/257 zero).
    # slot t of parity j holds row 2t+j.
    x_in = nc.dram_tensor("x_in", [128, NPAIR, SLOT], BF16,
                          kind="ExternalInput").ap()
    # small inputs all padded to 128 partitions: a DMA spanning fewer
    # partitions leaves empty SDMA engine-slots whose semaphore increments
    # can straggle by 5-12us, gating the whole DFT chain. Logits are
    # host-replicated to all partitions so sigmoid directly yields the
    # per-partition att scalars (no PE broadcast needed).
    lg_in = nc.dram_tensor("lg_in", [128, KNUM], F32, kind="ExternalInput").ap()
    pka_in = nc.dram_tensor("pka_in", [128, _PKAW], BF16, kind="ExternalInput").ap()
    pkb_in = nc.dram_tensor("pkb_in", [128, _PKBW], BF16, kind="ExternalInput").ap()
    ab_in = nc.dram_tensor("ab_in", [128, 6 * 64], BF16, kind="ExternalInput").ap()
    # device-side output layout: partition (r*64+co); plane r=0 slot s =
    # row 2s-1, plane r=1 slot s = row 2s. Plane 0 slot 0 and plane 1 slot
    # 128 are never written with valid data; host ignores them. Every store
    # group is then one [128, G*W] DMA.
    out = nc.dram_tensor("out", [128, NPAIR + 1, W], BF16,
                         kind="ExternalOutput").ap()

    with ExitStack() as ctx:
        cpool = ctx.enter_context(tc.tile_pool(name="cpool", bufs=1))
        xbpool = ctx.enter_context(tc.tile_pool(name="xbpool", bufs=8))
        spool = ctx.enter_context(tc.tile_pool(name="spool", bufs=3))
        # ONE unified PSUM pool (8 x [128,512] banks) shared by warmup, DFT
        # and conv accumulators: the shared rotation makes the conv's first
        # tiles land on the earliest-freed banks instead of colliding with
        # the last T copy (a ~1.4us WAR stall otherwise).
        cps = ctx.enter_context(tc.tile_pool(name="ps", bufs=8, space="PSUM"))

        # ---- PE warmup: HAM releases the clock gate only after ~3.4us of
        # sustained PE activity; without this the DFT chain and the first
        # ~3us of the conv run at 1.2 GHz. Dummy matmuls on zeroed tiles
        # keep the PE busy from kernel start until the real work (gated on
        # the constant-pack DMA) is ready.
        warm_w = cpool.tile([128, 128], BF16, name="warm_w")
        warm_x = cpool.tile([128, 512], BF16, name="warm_x")
        nc.gpsimd.memset(warm_w[:], 0.0)
        nc.gpsimd.memset(warm_x[:], 0.0)
        warm_ps = cps.tile([128, 512], F32, name="pair_ps")
        for _ in range(11):
            nc.tensor.matmul(warm_ps[:], warm_w[:], warm_x[:],
                             start=True, stop=True)

        # ---- small input loads, one DMA each, all on the sync (HWDGE)
        # queue so the scalar queue stays free for the ACT table loads the
        # sigmoid needs. logits first: the att path gates the premix.
        # one latency-critical load per queue: pkA (spectrum, gates premix)
        # alone on sync; pkB (basis) on scalar; the 16B logits on the idle
        # gpsimd SWDGE queue so nothing queues ahead of the sigmoid.
        # pkA split in two DMAs: the attD chain starts on the dre half as
        # soon as it lands instead of waiting for the whole pack's sem
        pka_sb = cpool.tile([128, _PKAW], BF16, name="pka_sb")
        nc.sync.dma_start(out=pka_sb[:, 0:8 * D2R], in_=pka_in[:, 0:8 * D2R])
        nc.sync.dma_start(out=pka_sb[:, 8 * D2R:_PKAW],
                          in_=pka_in[:, 8 * D2R:_PKAW])
        ab_sb = cpool.tile([128, 6 * 64], BF16, name="ab_sb")
        nc.sync.dma_start(out=ab_sb[:], in_=ab_in)
        l_sb = cpool.tile([128, KNUM], F32, name="l_sb")
        nc.gpsimd.dma_start(out=l_sb[:], in_=lg_in)
        pkb_sb = cpool.tile([128, _PKBW], BF16, name="pkb_sb")
        nc.scalar.dma_start(out=pkb_sb[:], in_=pkb_in)
        att_sb = cpool.tile([128, KNUM], F32, name="att_sb")
        sig_op = nc.scalar.activation(att_sb[:], l_sb[:],
                                      mybir.ActivationFunctionType.Sigmoid)

        dre_sb = pka_sb[0:96, 0:8 * D2R]
        dim_sb = pka_sb[0:96, 8 * D2R:2 * 8 * D2R]
        cb_sb = pkb_sb[0:96, 0:512]
        sb_sb = pkb_sb[0:96, 512:1024]
        sn_sb = pkb_sb[0:96, 1024:1536]

        gtre_sb = cpool.tile([D2R, 256], BF16, name="gtre_sb")
        gtim_sb = cpool.tile([D2R, 256], BF16, name="gtim_sb")
        t_sb = [cpool.tile([128, 128], BF16, name=f"t_sb_{i}") for i in range(6)]

        premix_first = None
        if True:
            # ---- att-scaled spectrum chunks on DVE (fast tensor_scalar
            # class); k-interleaved emission so stage-1 matmuls pipeline
            # behind them and the PE never idles long enough to re-throttle.
            attDre = cpool.tile([96, 8 * D2R], BF16, name="attDre")
            attDim = cpool.tile([96, 8 * D2R], BF16, name="attDim")
            premix_first = None
            for k in range(KNUM):
                o = nc.vector.tensor_scalar_mul(
                    attDre[:, k * 2 * D2R:(k + 1) * 2 * D2R],
                    dre_sb[:, k * 2 * D2R:(k + 1) * 2 * D2R],
                    att_sb[0:96, k:k + 1])
                if premix_first is None:
                    premix_first = o
                nc.vector.tensor_scalar_mul(
                    attDim[:, k * 2 * D2R:(k + 1) * 2 * D2R],
                    dim_sb[:, k * 2 * D2R:(k + 1) * 2 * D2R],
                    att_sb[0:96, k:k + 1])

            # ---- stage 1: GT_re = sum_k [aDre_k^T.C - aDim_k^T.S],
            #              GT_im = sum_k [aDre_k^T.S + aDim_k^T.C]
            # (contraction over (k, h); gtim group first so its copy
            # overlaps the gtre matmuls and stage 2 opens without a stall)
            gtim_ps = cps.tile([128, 512], F32, name="pair_ps")[0:D2R, 0:256]
            gtre_ps = cps.tile([128, 512], F32, name="pair_ps")[0:D2R, 0:256]

            def adre(k, h):
                return attDre[:, k * 2 * D2R + h * D2R: k * 2 * D2R + (h + 1) * D2R]

            def adim(k, h):
                return attDim[:, k * 2 * D2R + h * D2R: k * 2 * D2R + (h + 1) * D2R]

            for k in range(KNUM):
                for h in range(2):
                    nc.tensor.matmul(gtim_ps[:], adre(k, h),
                                     sb_sb[:, h * 256:(h + 1) * 256],
                                     start=(k == 0 and h == 0), stop=False)
                    nc.tensor.matmul(gtim_ps[:], adim(k, h),
                                     cb_sb[:, h * 256:(h + 1) * 256],
                                     start=False, stop=(k == 3 and h == 1))
            for k in range(KNUM):
                for h in range(2):
                    nc.tensor.matmul(gtre_ps[:], adre(k, h),
                                     cb_sb[:, h * 256:(h + 1) * 256],
                                     start=(k == 0 and h == 0), stop=False)
                    nc.tensor.matmul(gtre_ps[:], adim(k, h),
                                     sn_sb[:, h * 256:(h + 1) * 256],
                                     start=False, stop=(k == 3 and h == 1))
            nc.vector.tensor_copy(gtim_sb[:], gtim_ps[:])
            nc.scalar.copy(gtre_sb[:], gtre_ps[:])

            # ---- stage 2: six conv weight matrices T[(ci,j),(co,r)],
            # 2 quadrant-merged matmul pairs each (rhs windows contiguous
            # thanks to the phase-block basis layout)
            t_copy = None
            for i, (which, dx) in enumerate(_WSEQ):
                t_ps = cps.tile([128, 512], F32, name="pair_ps")[:, 0:128]
                zj, zr = _ZQUAD[which]
                nc.vector.memset(t_ps[64 * zj:64 * zj + 64, 64 * zr:64 * zr + 64], 0.0)
                for (J, o0, o1, c0, c1) in _S2PLAN[which]:
                    o = t_ps[64 * J:64 * J + 64, o0:o1]
                    nc.tensor.matmul(o, ab_sb[0:D2R, (3 + dx) * 64:(4 + dx) * 64],
                                     gtim_sb[:, c0:c1], start=True, stop=False)
                    nc.tensor.matmul(o, ab_sb[0:D2R, dx * 64:(dx + 1) * 64],
                                     gtre_sb[:, c0:c1], start=False, stop=True)
                if i % 2 == 0:
                    t_copy = nc.vector.tensor_copy(t_sb[2 * dx + which][:], t_ps[:])
                else:
                    t_copy = nc.scalar.copy(t_sb[2 * dx + which][:], t_ps[:])

        # ---- x chunk loads: slot t holds rows (2t, 2t+1). One 128-partition
        # DMA per chunk engages all 16 SDMA engines. All chunks wait for the
        # small latency-critical loads: 0-1 until the logits landed
        # (sigmoid), 2+ until the dft chain is done (t_copy).
        xch = []
        for c in range(NCH):
            # chunks hold 17 slots (1-slot overlap) so 2-pair windows never
            # cross a tile boundary; the last chunk has no slot 128.
            # chunks 0-1 ride the (idle) sync queue so the gpsimd queue is
            # free for the premix im-chain.
            nsl = CHS + 1 if c + 1 < NCH else CHS
            xb = xbpool.tile([128, (CHS + 1) * SLOT], BF16, name="xb")
            eng = nc.sync if c < 2 else nc.gpsimd
            di = eng.dma_start(
                out=xb[:, 0:nsl * SLOT],
                in_=x_in[:, c * CHS:c * CHS + nsl, :])
            bass._add_dep_helper(di.ins, premix_first.ins,
                                 reason="x chunks yield HBM to small loads")
            xch.append(xb)

        def slot_rhs(s, dx, npair=1):
            # [128, npair, W] window starting at slot s (npair<=2; both slots
            # live in chunk s//CHS thanks to the 1-slot overlap)
            c, loc = s // CHS, s % CHS
            if npair == 1:
                return xch[c][:, loc * SLOT + dx: loc * SLOT + dx + W]
            v = xch[c].rearrange("p (t s) -> p t s", s=SLOT)
            return v[:, loc:loc + npair, dx:dx + W]

        # ---- conv over row pairs
        # store groups over slots s in [0, 129): slot s = pair u = s-1
        # (slot 0 holds only row 0 on partitions 64-127; slot 128 only row
        # 255 on partitions 0-63). Big groups early, small at the end so
        # the last stores drain quickly.
        gsizes = [17] + [16] * 6 + [8, 4, 2, 1, 1]
        gstart = np.cumsum([0] + gsizes).tolist()

        def group_of(s):
            for gi in range(len(gsizes)):
                if s < gstart[gi + 1]:
                    return gi, s - gstart[gi]
            raise AssertionError

        # units: (-1,) special, (0,1), (2,3), ..., (124,125), (126,), (127,)
        units = [(-1,)] + [(u, u + 1) for u in range(0, 126, 2)] + [(126,), (127,)]

        if True:
            staging = {}

            def get_staging(gi):
                if gi not in staging:
                    if gi >= 7:
                        # small late groups get dedicated slots so the final
                        # copies never wait on store completions
                        staging[gi] = spool.tile(
                            [128, gsizes[gi] * W], BF16,
                            name=f"staging_l{gi}", bufs=1)
                    else:
                        staging[gi] = spool.tile(
                            [128, 17 * W], BF16, name="staging")
                return staging[gi]

            def unit_mms(un):
                L = []
                for wh, dx in _WSEQ:
                    if wh == 0 and un[0] < 0:
                        continue
                    if wh == 1 and un[0] > 126:
                        continue
                    L.append((wh, dx))
                return L

            def emit_block(uns):
                tiles = {}
                for un in uns:
                    tiles[un] = cps.tile([128, len(un) * W], F32, name="pair_ps")
                plan = {un: unit_mms(un) for un in uns}
                for k, (wh, dx) in enumerate(_WSEQ):
                    for un in uns:
                        if (wh, dx) not in plan[un]:
                            continue
                        i = plan[un].index((wh, dx))
                        rhs = slot_rhs(un[0] + (0 if wh == 0 else 1), dx,
                                       len(un))
                        nc.tensor.matmul(
                            tiles[un][:], t_sb[2 * dx + wh][:], rhs,
                            start=(i == 0), stop=(i == len(plan[un]) - 1),
                            skip_group_check=True)
                for un in uns:
                    for j, u in enumerate(un):
                        gi, si = group_of(u + 1)
                        st = get_staging(gi)[:, si * W:(si + 1) * W]
                        src = tiles[un][:, j * W:(j + 1) * W]
                        if u == -1:
                            nc.scalar.copy(st[64:128, :], src[64:128, :])
                        elif u == 127:
                            nc.scalar.copy(st[0:64, :], src[0:64, :])
                        elif (j == 0 and len(un) == 2 and
                              group_of(un[1] + 1)[0] == gi):
                            # both halves land in the same staging tile: one
                            # wide copy, alternating engines per unit
                            st2 = get_staging(gi)[:, si * W:(si + 2) * W]
                            if (u // 2) % 2 == 0:
                                nc.vector.tensor_copy(st2, tiles[un][:])
                            else:
                                nc.scalar.copy(st2, tiles[un][:])
                            break
                        elif u % 2 == 0:
                            nc.vector.tensor_copy(st, src)
                        else:
                            nc.scalar.copy(st, src)
                    for u in un:
                        gi, si = group_of(u + 1)
                        if si == gsizes[gi] - 1:
                            emit_stores(gi)

            store_cnt = [0]
            store_engs = [nc.sync, nc.gpsimd, nc.scalar]

            def emit_stores(gi):
                stg = staging.pop(gi)
                s0, s1 = gstart[gi], gstart[gi + 1]
                G = s1 - s0
                if gi >= 7:
                    # keep the scalar queue clear near the end: it does the
                    # final PSUM->SBUF copies that gate the last stores
                    eng = nc.sync if gi % 2 else nc.gpsimd
                else:
                    eng = store_engs[store_cnt[0] % 3]
                    store_cnt[0] += 1
                eng.dma_start(out=out[:, s0:s1, :], in_=stg[:, 0:G * W])

            # blocks of up to 4 units
            ui = 0
            while ui < len(units):
                emit_block(units[ui:ui + 4])
                ui += 4


_NC_CACHE = None


def _build_nc():
    global _NC_CACHE
    if _NC_CACHE is None:
        nc = bacc.Bacc("TRN2", target_bir_lowering=False, debug=False,
                       num_devices=B)
        with tile.TileContext(nc) as tc:
            _emit_kernel(tc)
        nc.compile()
        _NC_CACHE = nc
    return _NC_CACHE


def _in_maps(x, k_att_logits, dft_weight):
    x = np.asarray(x, dtype=np.float32)
    lg = np.asarray(k_att_logits, dtype=np.float32)
    dw = np.asarray(dft_weight, dtype=np.float32)

    # x -> bf16, parity-split rows, host-inserted zero pad columns
    xp = np.zeros((B, 2, CIN, NPAIR, SLOT), dtype=ml_dtypes.bfloat16)
    xv = x.reshape(B, CIN, NPAIR, 2, W).transpose(0, 3, 1, 2, 4)  # [b,j,c,t,w]
    xp[:, :, :, :, 1:1 + W] = xv.astype(ml_dtypes.bfloat16)
    xp = xp.reshape(B, 128, NPAIR, SLOT)

    # host-side gather: dense half-spectrum layout [k, h, w, c], chunk-major
    dftP = dw[:, _INV, :].reshape(KNUM, 2, 96, D2R, 2)   # [k, half, p, w, c]
    dre = dftP[..., 0].transpose(2, 0, 1, 3).reshape(96, 8 * D2R)
    dim = dftP[..., 1].transpose(2, 0, 1, 3).reshape(96, 8 * D2R)
    pka = np.zeros((128, _PKAW), dtype=ml_dtypes.bfloat16)
    pka[:96, 0:8 * D2R] = dre.astype(ml_dtypes.bfloat16)
    pka[:96, 8 * D2R:2 * 8 * D2R] = dim.astype(ml_dtypes.bfloat16)
    pkb = np.zeros((128, _PKBW), dtype=ml_dtypes.bfloat16)
    pkb[:96, 0:512] = _C2.astype(ml_dtypes.bfloat16)
    pkb[:96, 512:1024] = _S2.astype(ml_dtypes.bfloat16)
    pkb[:96, 1024:1536] = (-_S2).astype(ml_dtypes.bfloat16)
    abp = np.zeros((128, 6 * 64), dtype=ml_dtypes.bfloat16)
    abp[:D2R] = _AB
    # logits replicated to all partitions: sigmoid then yields per-partition
    # att directly, no PE broadcast
    lgp = np.broadcast_to(lg[:, None, :], (B, 128, KNUM))

    maps = []
    for b in range(B):
        maps.append({
            "x_in": np.ascontiguousarray(xp[b]),
            "lg_in": np.ascontiguousarray(lgp[b]),
            "pka_in": pka,
            "pkb_in": pkb,
            "ab_in": abp,
        })
    return maps


def _execute(x, k_att_logits, dft_weight, trace=False, **trace_kwargs):
    nc = _build_nc()
    res = run_bass_kernel_spmd(
        nc, _in_maps(x, k_att_logits, dft_weight),
        core_ids=list(range(B)), trace=trace, **trace_kwargs)
    out = np.empty((B, COUT, H, W), dtype=np.float32)
    for b in range(B):
        dev = np.asarray(res.results[b]["out"]).reshape(2, COUT, NPAIR + 1, W)
        out[b, :, 1::2, :] = dev[0, :, 1:NPAIR + 1, :].astype(np.float32)
        out[b, :, 0::2, :] = dev[1, :, 0:NPAIR, :].astype(np.float32)
    return out, res


def kernel(x, k_att_logits, dft_weight):
    out, _ = _execute(x, k_att_logits, dft_weight)
    return out.astype(np.float32)
